# revision 23
# baseline (speedup 1.0000x reference)
"""TRN2 Bass kernel for nn_DeeperGCNLayerMix (GENConv softmax-aggr + MLP/BN/LN mix).

Self-contained: accepts FULL inputs, shards nodes across 8 NeuronCores
internally (SPMD, one NEFF), returns the FULL [50000, 128] output.

v2 strategy (vs v1 baseline):
- Nodes sharded by dst range across 8 cores. Per-core edges bucketed by
  128-node dst window and sorted by src within each window; chunks of 128
  edges. Gather calls are layer-major across a superblock of 7 windows:
  call j gathers the j-th chunk of each window. All chunks in a call share
  one compile-time base row offset into x, with int16 indices relative to
  the base (max span ~16k < 32768) - this removes the lo/hi class split.
- dma_gather descriptor generation is the bottleneck (Q7 core pairs,
  ~8ns/descriptor). Calls round-robin the 4 SWDGE queues, which map to
  disjoint Q7 core pairs, giving ~4x parallel descriptor generation.
  num_idxs registers are materialized once per distinct count so no
  per-call MOVE serializes the gpsimd stream; gather buffers are deep
  (bufs=8) so buffer-reuse waits are pre-satisfied.
- Edge math: r = relu(t*g) (DVE two-op tensor_scalar), e = exp(r) (ACT),
  u = r*e (DVE fp16), one-hot via is_equal (DVE fp16). Two fp16 matmuls
  per chunk accumulate [ch, (s|u)] into a superblock-wide PSUM tile
  [128, 7*256]; one ACT copy per superblock into swT.
- Softmax shift invariance removes the segment-max pass; t is folded into
  r (u' = t*u) and folded back in the denominator reciprocal.
- Node phase ch-major: h = u'/(t*(s+1e-16)) + x + eps, h@W1 (fp16), global
  BatchNorm stats via ACT accum_out sums + AllReduce of [128,4] partials,
  fused affine+relu (ACT), @W2 (fp16), PE transpose back to node-major,
  LayerNorm per node, mixed activation + residual, DMA out.
"""

from contextlib import ExitStack
from dataclasses import dataclass, field

import numpy as np

import concourse.bacc as bacc
import concourse.mybir as mybir
import concourse.tile as tile
from concourse import bass_utils

F32 = mybir.dt.float32
F16 = mybir.dt.float16
I16 = mybir.dt.int16
AF = mybir.ActivationFunctionType
ALU = mybir.AluOpType

N = 50000
NC = 8
D = 128
W = 128
SBW = 7
NT = 512
EPS_MSG = 1e-7
BN_EPS = 1e-5
LN_EPS = 1e-5
BETA_L = 0.5
NQ = 4  # SWDGE queues to round-robin (1..4)


@dataclass
class Plan:
    N: int
    NSH: int = 0
    NW: int = 0
    NPAD: int = 0
    CT: int = 0
    nch: list = field(default_factory=list)
    # calls: list of dicts {sb, j, g0, k, base, rows, chunks:[(w, last)]}
    calls: list = field(default_factory=list)

    def key(self):
        return (self.N, tuple(self.nch),
                tuple((c["g0"], c["k"], c["base"], c["rows"]) for c in self.calls))


def make_plan(n, edge_index):
    src = np.asarray(edge_index[0]).astype(np.int64)
    dst = np.asarray(edge_index[1]).astype(np.int64)
    p = Plan(N=n)
    p.NSH = n // NC
    p.NW = (p.NSH + W - 1) // W
    p.NPAD = p.NW * W

    core = dst // p.NSH
    win = (dst % p.NSH) // W
    cnt = np.zeros((NC, p.NW), np.int64)
    np.add.at(cnt, (core, win), 1)
    nch = np.ceil(cnt / 128).astype(np.int64).max(axis=0)
    nch = np.maximum(nch, 1)
    p.nch = nch.tolist()
    p.CT = int(nch.sum())

    # per-core sorted srcs per window to compute call bases/spans
    wstarts, wends, s_sorted = [], [], []
    for c in range(NC):
        m = core == c
        s_c, w_c = src[m], win[m]
        order = np.lexsort((s_c, w_c))
        s_s, w_s = s_c[order], w_c[order]
        s_sorted.append(s_s)
        wstarts.append(np.searchsorted(w_s, np.arange(p.NW)))
        wends.append(np.searchsorted(w_s, np.arange(p.NW) + 1))

    g0 = 0
    nsb = (p.NW + SBW - 1) // SBW
    for sb in range(nsb):
        ws = list(range(sb * SBW, min((sb + 1) * SBW, p.NW)))
        L = max(nch[w] for w in ws)
        for j0 in range(0, L, 2):
            # merge two chunk layers into one gather call
            chunks = [(w, j, j == nch[w] - 1)
                      for j in (j0, j0 + 1) if j < L
                      for w in ws if j < nch[w]]
            k = len(chunks)
            lo, hi = 1 << 60, -1
            for c in range(NC):
                for (w, j, _) in chunks:
                    a = wstarts[c][w] + 128 * j
                    b = min(wstarts[c][w] + 128 * (j + 1), wends[c][w])
                    if a < b:
                        lo = min(lo, int(s_sorted[c][a]))
                        hi = max(hi, int(s_sorted[c][b - 1]))
            assert hi >= 0, (sb, j0)
            assert hi - lo < 32768, (sb, j0, lo, hi)
            p.calls.append(dict(sb=sb, g0=g0, k=k, base=lo,
                                rows=hi - lo + 1, chunks=chunks))
            g0 += k
    assert g0 == p.CT
    return p


def make_core_inputs(p, x, edge_index, t, W1, b1, bn_gamma, bn_beta,
                     W2, b2, ln_gamma, ln_beta):
    x = np.ascontiguousarray(np.asarray(x, np.float32))
    src = np.asarray(edge_index[0]).astype(np.int64)
    dst = np.asarray(edge_index[1]).astype(np.int64)

    iota = np.broadcast_to(np.arange(128, dtype=np.float16), (128, 128)).copy()
    ident = np.eye(128, dtype=np.float32)
    lngh = np.broadcast_to(
        ((1.0 - BETA_L) * np.asarray(ln_gamma, np.float32)).astype(np.float16),
        (128, 128)).copy()
    lnbh = np.broadcast_to(
        ((1.0 - BETA_L) * np.asarray(ln_beta, np.float32)).astype(np.float16),
        (128, 128)).copy()

    tval = float(np.asarray(t))
    vecs = np.zeros((128, 8), np.float32)
    vecs[:, 0] = tval
    vecs[:, 1] = np.asarray(b2, np.float32)
    vecs[:, 2] = np.asarray(bn_gamma, np.float32)[0:128]
    vecs[:, 3] = np.asarray(bn_gamma, np.float32)[128:256]
    vecs[:, 4] = np.asarray(bn_beta, np.float32)[0:128]
    vecs[:, 5] = np.asarray(bn_beta, np.float32)[128:256]
    vecs[:, 6] = EPS_MSG
    vecs[:, 7] = tval * 1e-16

    W1m = np.ascontiguousarray(np.asarray(W1, np.float32).astype(np.float16))
    W2m = np.ascontiguousarray(np.asarray(W2, np.float32).astype(np.float16))

    core = dst // p.NSH
    in_maps = []
    for c in range(NC):
        m = core == c
        s_c = src[m]
        d_c = dst[m] - c * p.NSH
        w_c = d_c // W
        slot_c = (d_c % W).astype(np.float16)
        order = np.lexsort((s_c, w_c))
        s_s, w_s = s_c[order], w_c[order]
        slot_s = slot_c[order]
        wstart = np.searchsorted(w_s, np.arange(p.NW))
        wend = np.searchsorted(w_s, np.arange(p.NW) + 1)

        ids = np.zeros(p.CT * 128, np.int16)
        dstloc = np.full((128, p.CT), -1.0, np.float16)
        for call in p.calls:
            base = call["base"]
            pos = call["g0"] * 128
            for ci, (w, j, _) in enumerate(call["chunks"]):
                g = call["g0"] + ci
                a = wstart[w] + 128 * j
                b = min(wstart[w] + 128 * (j + 1), wend[w])
                nreal = max(0, b - a)
                if nreal > 0:
                    ids[pos:pos + nreal] = (s_s[a:b] - base).astype(np.int16)
                    dstloc[0:nreal, g] = slot_s[a:b]
                pos += 128

        a = ids.reshape(-1, 16).T
        idx_w = np.tile(a, (8, 1)).copy()

        lo_n, hi_n = c * p.NSH, (c + 1) * p.NSH
        im = {
            "x": x,
            "xshard": np.pad(x[lo_n:hi_n], ((0, p.NPAD - p.NSH), (0, 0))),
            "idx": idx_w,
            "dstloc": dstloc,
            "iota": iota,
            "ident": ident,
            "W1f16": W1m,
            "W2f16": W2m,
            "vecs": vecs,
            "lngh": lngh,
            "lnbh": lnbh,
        }
        in_maps.append(im)
    return in_maps


def input_specs(p):
    return {
        "x": ([p.N, D], F32),
        "xshard": ([p.NPAD, D], F32),
        "idx": ([128, p.CT * 8], I16),
        "dstloc": ([128, p.CT], F16),
        "iota": ([128, 128], F16),
        "ident": ([128, 128], F32),
        "W1f16": ([128, 256], F16),
        "W2f16": ([256, 128], F16),
        "vecs": ([128, 8], F32),
        "lngh": ([128, 128], F16),
        "lnbh": ([128, 128], F16),
    }


def emit_kernel(ctx, tc, p, aps):
    nc = tc.nc
    NPAD, NW, NSH = p.NPAD, p.NW, p.NSH
    nsb = (NW + SBW - 1) // SBW

    cpool = ctx.enter_context(tc.tile_pool(name="consts", bufs=1))
    idxt = cpool.tile([128, p.CT * 8], I16, tag="idx")
    nc.sync.dma_start(idxt[:], aps["idx"][:])
    dstloc = cpool.tile([128, p.CT], F16, tag="dstloc")
    nc.sync.dma_start(dstloc[:], aps["dstloc"][:])
    iota = cpool.tile([128, 128], F16, tag="iota")
    nc.sync.dma_start(iota[:], aps["iota"][:])
    ident = cpool.tile([128, 128], F32, tag="ident")
    nc.sync.dma_start(ident[:], aps["ident"][:])
    W1t = cpool.tile([128, 256], F16, tag="w1")
    nc.sync.dma_start(W1t[:], aps["W1f16"][:])
    W2t = [cpool.tile([128, 128], F16, tag=f"w2_{i}", name=f"w2t_{i}")
           for i in range(2)]
    nc.sync.dma_start(W2t[0][:], aps["W2f16"][0:128, :])
    nc.sync.dma_start(W2t[1][:], aps["W2f16"][128:256, :])
    vecs = cpool.tile([128, 8], F32, tag="vecs")
    nc.sync.dma_start(vecs[:], aps["vecs"][:])
    lngh = cpool.tile([128, 128], F16, tag="lngh")
    nc.sync.dma_start(lngh[:], aps["lngh"][:])
    lnbh = cpool.tile([128, 128], F16, tag="lnbh")
    nc.sync.dma_start(lnbh[:], aps["lnbh"][:])
    t_ap = vecs[:, 0:1]
    b2_ap = vecs[:, 1:2]

    xT = cpool.tile([128, NPAD], F32, tag="xT")
    hf = cpool.tile([128, NPAD], F16, tag="hf")

    swpool = tc.tile_pool(name="swp", bufs=1)
    swp = swpool.__enter__()
    swT = swp.tile([128, NW * 256], F32, tag="swT")

    # ---- xT: transposed (ch-major) x + eps, built before edge PSUM opens ----
    with tc.tile_pool(name="tp0", bufs=2, space="PSUM") as tp0, \
         tc.tile_pool(name="sp0", bufs=4) as sp0:
        w = 0
        while w < NW:
            gw = min(4, NW - w)
            ps = tp0.tile([128, 512], F32, tag="pst")
            for i in range(gw):
                xin = sp0.tile([128, 128], F32, tag="xin")
                nc.sync.dma_start(
                    xin[:], aps["xshard"][(w + i) * 128:(w + i + 1) * 128, :])
                nc.tensor.transpose(ps[:, i * 128:(i + 1) * 128], xin[:],
                                    ident[:])
            nc.scalar.activation(xT[:, w * 128:(w + gw) * 128],
                                 ps[:, 0:gw * 128], AF.Identity,
                                 bias=vecs[:, 6:7], scale=1.0)
            w += gw

    # ---- edge phase ----
    kset = sorted({c["k"] for c in p.calls})
    regs = {k: nc.gpsimd.to_reg(k * 128) for k in kset}

    KW = 2 * SBW
    with tc.tile_pool(name="gat", bufs=8) as gp, \
         tc.tile_pool(name="vals", bufs=3) as vp, \
         tc.tile_pool(name="epsum", bufs=8, space="PSUM") as pp:
        qn = 0
        psw = {}
        sb_calls = {}
        for call in p.calls:
            sb_calls.setdefault(call["sb"], []).append(call)
        for sb in range(nsb):
            for call in sb_calls[sb]:
                k, g0 = call["k"], call["g0"]
                g = gp.tile([128, KW, 128], F32, tag="g")
                nc.gpsimd.dma_gather(
                    g[:, 0:k, :],
                    aps["x"][call["base"]:call["base"] + call["rows"], :],
                    idxt[:, g0 * 8:(g0 + k) * 8],
                    num_idxs=k * 128, num_idxs_reg=regs[k], elem_size=128,
                    single_packet=False, queue_num=qn)
                qn = (qn + 1) % NQ
                gf = g[:, 0:k, :].rearrange("p a b -> p (a b)")
                r = vp.tile([128, KW, 128], F16, tag="r")
                rf = r[:, 0:k, :].rearrange("p a b -> p (a b)")
                nc.scalar.activation(rf, gf, AF.Relu, scale=t_ap)
                e = vp.tile([128, KW, 128], F16, tag="e")
                ef = e[:, 0:k, :].rearrange("p a b -> p (a b)")
                nc.scalar.activation(ef, rf, AF.Exp)
                u = vp.tile([128, KW, 128], F16, tag="u")
                uf = u[:, 0:k, :].rearrange("p a b -> p (a b)")
                nc.vector.tensor_tensor(uf, rf, ef, op=ALU.mult)
                oh = vp.tile([128, KW, 128], F16, tag="oh")
                nc.vector.tensor_tensor(
                    oh[:, 0:k, :],
                    iota[:].unsqueeze(1).broadcast_to([128, k, 128]),
                    dstloc[:, g0:g0 + k].unsqueeze(2).broadcast_to(
                        [128, k, 128]),
                    op=ALU.is_equal)
                for ci, (w, j, last) in enumerate(call["chunks"]):
                    st = j == 0
                    if st:
                        psw[w] = pp.tile([128, 256], F32, tag="ps",
                                         name=f"psw_{w}")
                    nc.tensor.matmul(psw[w][:, 0:128],
                                     e[:, ci, :], oh[:, ci, :],
                                     start=st, stop=last,
                                     skip_group_check=True)
                    nc.tensor.matmul(psw[w][:, 128:256],
                                     u[:, ci, :], oh[:, ci, :],
                                     start=False, stop=last,
                                     skip_group_check=True)
                    if last:
                        nc.scalar.copy(swT[:, w * 256:(w + 1) * 256],
                                       psw[w][:])
                        del psw[w]

    if "dbg_sw" in aps:
        nc.sync.dma_start(aps["dbg_sw"][:], swT[:])

    # ---- aggregation: agg = u' / (t*(s+1e-16)); hf = agg + x + eps ----
    swv = swT[:].rearrange("p (w q) -> p w q", q=256)
    with tc.tile_pool(name="agg", bufs=1) as agp:
        A = agp.tile([128, NW, 128], F32, tag="A")
        nc.vector.tensor_scalar(A[:], swv[:, :, 0:128], t_ap, vecs[:, 7:8],
                                ALU.mult, ALU.add)
        rcp = agp.tile([128, NW, 128], F32, tag="B")
        nc.vector.reciprocal_approx_fast(rcp[:], A[:])
        nc.vector.tensor_tensor(A[:], swv[:, :, 128:256], rcp[:], op=ALU.mult)
        af = A[:].rearrange("p w q -> p (w q)")
        nc.vector.tensor_tensor(hf[:], af, xT[:], op=ALU.add)

    if "dbg_hf" in aps:
        nc.sync.dma_start(aps["dbg_hf"][:], hf[:])

    # swT no longer needed; free its pool (LIFO: agg pool already closed)
    swpool.__exit__(None, None, None)

    # ---- node phase ----
    np3 = ctx.enter_context(tc.tile_pool(name="node3", bufs=1))
    dramp = ctx.enter_context(tc.tile_pool(name="dram", bufs=1, space="DRAM"))

    with tc.tile_pool(name="tpsum", bufs=2, space="PSUM") as tp, \
         tc.tile_pool(name="scr", bufs=2) as sp:
        # ---- h1 = hf @ W1 (fp16), with BN partial sums via accum_out ----
        h1 = [np3.tile([128, NPAD], F16, tag=f"H{i}", name=f"h1_{i}")
              for i in range(2)]
        ntiles = []
        o = 0
        while o < NPAD:
            ntiles.append((o, min(NT, NPAD - o)))
            o += NT
        nacc = len(ntiles) + 1
        acc = sp.tile([128, 2 * nacc], F32, tag="acc")
        scratch = np3.tile([128, NPAD], F16, tag="Z")
        for ch in (0, 1):
            for i, (o, sz) in enumerate(ntiles):
                psm = tp.tile([128, NT], F32, tag="psmm")
                nc.tensor.matmul(psm[:, 0:sz], W1t[:, ch * 128:(ch + 1) * 128],
                                 hf[:, o:o + sz], start=True, stop=True)
                if o + sz <= NSH:
                    nc.scalar.activation(h1[ch][:, o:o + sz], psm[:, 0:sz],
                                         AF.Identity,
                                         accum_out=acc[:, ch * nacc + i:
                                                       ch * nacc + i + 1])
                else:
                    # split: accumulate only real nodes into BN stats
                    real = NSH - o
                    assert real > 0
                    nc.scalar.activation(h1[ch][:, o:o + real],
                                         psm[:, 0:real], AF.Identity,
                                         accum_out=acc[:, ch * nacc + i:
                                                       ch * nacc + i + 1])
                    nc.scalar.copy(h1[ch][:, o + real:o + sz],
                                   psm[:, real:sz])
            # sum of squares over real nodes (one pass)
            nc.scalar.activation(scratch[:, 0:NSH], h1[ch][:, 0:NSH],
                                 AF.Square,
                                 accum_out=acc[:, ch * nacc + nacc - 1:
                                               ch * nacc + nacc])

        # partials: [sum0, sum1, sumsq0, sumsq1]
        partials = sp.tile([128, 4], F32, tag="partials")
        dump = sp.tile([128, nacc], F32, tag="dump")
        nc.scalar.activation(dump[:, 0:nacc - 1], acc[:, 0:nacc - 1],
                             AF.Identity, accum_out=partials[:, 0:1])
        nc.scalar.activation(dump[:, 0:nacc - 1], acc[:, nacc:2 * nacc - 1],
                             AF.Identity, accum_out=partials[:, 1:2])
        nc.vector.tensor_copy(partials[:, 2:3], acc[:, nacc - 1:nacc])
        nc.vector.tensor_copy(partials[:, 3:4], acc[:, 2 * nacc - 1:2 * nacc])

        ib = dramp.tile([128, 4], F32, tag="ib")
        ob = dramp.tile([128, 4], F32, tag="ob")
        nc.sync.dma_start(ib[:], partials[:])
        nc.gpsimd.collective_compute(
            "AllReduce", ALU.add, replica_groups=[list(range(NC))],
            ins=[ib[:].opt()], outs=[ob[:].opt()])
        gst = sp.tile([128, 4], F32, tag="gst")
        nc.sync.dma_start(gst[:], ob[:])

        mg = sp.tile([128, 2], F32, tag="mg")
        nc.vector.tensor_scalar(mg[:], gst[:, 0:2], 1.0 / N, None, ALU.mult)
        var = sp.tile([128, 2], F32, tag="var")
        nc.vector.tensor_tensor(var[:], mg[:], mg[:], op=ALU.mult)
        ex2 = sp.tile([128, 2], F32, tag="ex2")
        nc.vector.tensor_scalar(ex2[:], gst[:, 2:4], 1.0 / N, None, ALU.mult)
        nc.vector.tensor_tensor(var[:], ex2[:], var[:], op=ALU.subtract)
        nc.vector.tensor_scalar(var[:], var[:], float(BN_EPS), None, ALU.add)
        rcv = sp.tile([128, 2], F32, tag="rcv")
        nc.vector.reciprocal(rcv[:], var[:])
        rstd = sp.tile([128, 2], F32, tag="rstd")
        nc.scalar.sqrt(rstd[:], rcv[:])
        aaf = sp.tile([128, 2], F32, tag="aaf")
        nc.vector.tensor_tensor(aaf[:], vecs[:, 2:4], rstd[:], op=ALU.mult)
        baf = sp.tile([128, 2], F32, tag="baf")
        nc.vector.tensor_tensor(baf[:], mg[:], aaf[:], op=ALU.mult)
        nc.vector.tensor_tensor(baf[:], vecs[:, 4:6], baf[:], op=ALU.subtract)

        for ch in (0, 1):
            nc.scalar.activation(h1[ch][:], h1[ch][:], AF.Relu,
                                 bias=baf[:, ch:ch + 1], scale=aaf[:, ch:ch + 1])

        # ---- y = h1 @ W2 + b2 (ch-major), transpose to node-major ----
        yT = np3.tile([128, NPAD], F32, tag="A")
        for (o, sz) in ntiles:
            psm = tp.tile([128, NT], F32, tag="psy")
            nc.tensor.matmul(psm[:, 0:sz], W2t[0][:], h1[0][:, o:o + sz],
                             start=True, stop=False)
            nc.tensor.matmul(psm[:, 0:sz], W2t[1][:], h1[1][:, o:o + sz],
                             start=False, stop=True)
            nc.scalar.activation(yT[:, o:o + sz], psm[:, 0:sz], AF.Identity,
                                 bias=b2_ap, scale=1.0)

        yN = np3.tile([128, NPAD], F32, tag="B")
        mvall = sp.tile([128, NW * 2], F32, tag="mvall")
        w = 0
        while w < NW:
            gw = min(4, NW - w)
            psm = tp.tile([128, 512], F32, tag="psy")
            for i in range(gw):
                nc.tensor.transpose(psm[:, i * 128:(i + 1) * 128],
                                    yT[:, (w + i) * 128:(w + i + 1) * 128],
                                    ident[:])
            nc.scalar.copy(yN[:, w * 128:(w + gw) * 128], psm[:, 0:gw * 128])
            for i in range(gw):
                st6 = sp.tile([128, 6], F32, tag="st6")
                nc.vector.bn_stats(st6[:],
                                   yN[:, (w + i) * 128:(w + i + 1) * 128])
                nc.vector.bn_aggr(mvall[:, (w + i) * 2:(w + i + 1) * 2],
                                  st6[:])
            w += gw

        mvv = mvall[:].rearrange("p (w q) -> p w q", q=2)
        varn = sp.tile([128, NW], F32, tag="varn")
        nc.vector.tensor_scalar(varn[:], mvv[:, :, 1:2], float(LN_EPS), None,
                                ALU.add)
        rcn = sp.tile([128, NW], F32, tag="rcn")
        nc.vector.reciprocal(rcn[:], varn[:])
        rsn = sp.tile([128, NW], F32, tag="rsn")
        nc.scalar.sqrt(rsn[:], rcn[:])
        nmr = sp.tile([128, NW], F32, tag="nmr")
        nc.vector.tensor_tensor(nmr[:], mvv[:, :, 0:1].rearrange(
            "p w q -> p (w q)"), rsn[:], op=ALU.mult)
        nc.vector.tensor_scalar(nmr[:], nmr[:], -1.0, None, ALU.mult)

        # z = (yN - mu) * rstd per window (ACT per-partition scale/bias)
        Z = np3.tile([128, NPAD], F16, tag="Z")
        for w in range(NW):
            nc.scalar.activation(Z[:, w * 128:(w + 1) * 128],
                                 yN[:, w * 128:(w + 1) * 128], AF.Identity,
                                 bias=nmr[:, w:w + 1], scale=rsn[:, w:w + 1])
        # zh = 0.5*(gamma*z + beta); acc = zh + relu(zh); out = acc + 0.5*x
        zv = Z[:].rearrange("p (w q) -> p w q", q=128)
        nc.vector.tensor_tensor(zv, zv,
                                lngh[:].unsqueeze(1).broadcast_to(
                                    [128, NW, 128]), op=ALU.mult)
        nc.vector.tensor_tensor(zv, zv,
                                lnbh[:].unsqueeze(1).broadcast_to(
                                    [128, NW, 128]), op=ALU.add)
        RL = np3.tile([128, NPAD], F16, tag="R")
        nc.scalar.activation(RL[:], Z[:], AF.Relu)
        nc.vector.tensor_tensor(Z[:], Z[:], RL[:], op=ALU.add)
        xN = np3.tile([128, NPAD], F32, tag="A")
        nc.sync.dma_start(
            xN[:].rearrange("p (w q) -> p w q", q=128),
            aps["xshard"][:].rearrange("(w q) c -> q w c", q=128))
        xh = np3.tile([128, NPAD], F16, tag="R2")
        nc.vector.tensor_scalar(xh[:], xN[:], 1.0 - BETA_L, None, ALU.mult)
        out = np3.tile([128, NPAD], F32, tag="B")
        nc.vector.tensor_tensor(out[:], Z[:], xh[:], op=ALU.add)

        nc.sync.dma_start(
            aps["yout"][:].rearrange("(w q) c -> q w c", q=128),
            out[:].rearrange("p (w q) -> p w q", q=128))


_cache = {}


def _get_compiled(p, compile=True, dbg=False):
    key = (p.key(), compile, dbg)
    if key in _cache:
        return _cache[key]
    nc = bacc.Bacc("TRN2", target_bir_lowering=False, debug=False,
                   num_devices=NC, num_swdge_queues=4)
    aps = {}
    for name, (shape, dt) in input_specs(p).items():
        aps[name] = nc.dram_tensor(name, shape, dt, kind="ExternalInput").ap()
    aps["yout"] = nc.dram_tensor("yout", [p.NPAD, 128], F32,
                                 kind="ExternalOutput").ap()
    if dbg:
        aps["dbg_sw"] = nc.dram_tensor("dbg_sw", [128, p.NW * 256], F32,
                                       kind="ExternalOutput").ap()
        aps["dbg_hf"] = nc.dram_tensor("dbg_hf", [128, p.NPAD], F16,
                                       kind="ExternalOutput").ap()
    with tile.TileContext(nc) as tc:
        with ExitStack() as ctx:
            emit_kernel(ctx, tc, p, aps)
    if compile:
        nc.compile()
    _cache[key] = nc
    return nc


def kernel(x, edge_index, t, W1, b1, bn_gamma, bn_beta, W2, b2,
           ln_gamma, ln_beta):
    x = np.asarray(x)
    edge_index = np.asarray(edge_index)
    p = make_plan(x.shape[0], edge_index)
    ims = make_core_inputs(p, x, edge_index, t, W1, b1, bn_gamma, bn_beta,
                           W2, b2, ln_gamma, ln_beta)
    nc = _get_compiled(p)
    res = bass_utils.run_bass_kernel_spmd(nc, ims, core_ids=list(range(NC)))
    out = np.concatenate([res.results[c]["yout"][:p.NSH] for c in range(NC)])
    return out.astype(np.float32)


# revision 26
# speedup vs baseline: 1.0426x; 1.0426x over previous
"""TRN2 Bass kernel for nn_DeeperGCNLayerMix (GENConv softmax-aggr + MLP/BN/LN mix).

Self-contained: accepts FULL inputs, shards nodes across 8 NeuronCores
internally (SPMD, one NEFF), returns the FULL [50000, 128] output.

v2 strategy (vs v1 baseline):
- Nodes sharded by dst range across 8 cores. Per-core edges bucketed by
  128-node dst window and sorted by src within each window; chunks of 128
  edges. Gather calls are layer-major across a superblock of 7 windows:
  call j gathers the j-th chunk of each window. All chunks in a call share
  one compile-time base row offset into x, with int16 indices relative to
  the base (max span ~16k < 32768) - this removes the lo/hi class split.
- dma_gather descriptor generation is the bottleneck (Q7 core pairs,
  ~8ns/descriptor). Calls round-robin the 4 SWDGE queues, which map to
  disjoint Q7 core pairs, giving ~4x parallel descriptor generation.
  num_idxs registers are materialized once per distinct count so no
  per-call MOVE serializes the gpsimd stream; gather buffers are deep
  (bufs=8) so buffer-reuse waits are pre-satisfied.
- Edge math: r = relu(t*g) (DVE two-op tensor_scalar), e = exp(r) (ACT),
  u = r*e (DVE fp16), one-hot via is_equal (DVE fp16). Two fp16 matmuls
  per chunk accumulate [ch, (s|u)] into a superblock-wide PSUM tile
  [128, 7*256]; one ACT copy per superblock into swT.
- Softmax shift invariance removes the segment-max pass; t is folded into
  r (u' = t*u) and folded back in the denominator reciprocal.
- Node phase ch-major: h = u'/(t*(s+1e-16)) + x + eps, h@W1 (fp16), global
  BatchNorm stats via ACT accum_out sums + AllReduce of [128,4] partials,
  fused affine+relu (ACT), @W2 (fp16), PE transpose back to node-major,
  LayerNorm per node, mixed activation + residual, DMA out.
"""

from contextlib import ExitStack
from dataclasses import dataclass, field

import numpy as np

import concourse.bacc as bacc
import concourse.mybir as mybir
import concourse.tile as tile
from concourse import bass_utils

F32 = mybir.dt.float32
F16 = mybir.dt.float16
I16 = mybir.dt.int16
AF = mybir.ActivationFunctionType
ALU = mybir.AluOpType

N = 50000
NC = 8
D = 128
W = 128
SBW = 7
NT = 512
EPS_MSG = 1e-7
BN_EPS = 1e-5
LN_EPS = 1e-5
BETA_L = 0.5
NQ = 4  # SWDGE queues to round-robin (1..4)


@dataclass
class Plan:
    N: int
    NSH: int = 0
    NW: int = 0
    NPAD: int = 0
    CT: int = 0
    nch: list = field(default_factory=list)
    # calls: list of dicts {sb, j, g0, k, base, rows, chunks:[(w, last)]}
    calls: list = field(default_factory=list)

    def key(self):
        return (self.N, tuple(self.nch),
                tuple((c["g0"], c["k"], c["base"], c["rows"]) for c in self.calls))


def make_plan(n, edge_index):
    src = np.asarray(edge_index[0]).astype(np.int64)
    dst = np.asarray(edge_index[1]).astype(np.int64)
    p = Plan(N=n)
    p.NSH = n // NC
    p.NW = (p.NSH + W - 1) // W
    p.NPAD = p.NW * W

    core = dst // p.NSH
    win = (dst % p.NSH) // W
    cnt = np.zeros((NC, p.NW), np.int64)
    np.add.at(cnt, (core, win), 1)
    nch = np.ceil(cnt / 128).astype(np.int64).max(axis=0)
    nch = np.maximum(nch, 1)
    p.nch = nch.tolist()
    p.CT = int(nch.sum())

    # per-core sorted srcs per window to compute call bases/spans
    wstarts, wends, s_sorted = [], [], []
    for c in range(NC):
        m = core == c
        s_c, w_c = src[m], win[m]
        order = np.lexsort((s_c, w_c))
        s_s, w_s = s_c[order], w_c[order]
        s_sorted.append(s_s)
        wstarts.append(np.searchsorted(w_s, np.arange(p.NW)))
        wends.append(np.searchsorted(w_s, np.arange(p.NW) + 1))

    g0 = 0
    nsb = (p.NW + SBW - 1) // SBW
    for sb in range(nsb):
        ws = list(range(sb * SBW, min((sb + 1) * SBW, p.NW)))
        L = max(nch[w] for w in ws)
        for j0 in range(0, L, 2):
            # merge two chunk layers into one gather call
            chunks = [(w, j, j == nch[w] - 1)
                      for j in (j0, j0 + 1) if j < L
                      for w in ws if j < nch[w]]
            k = len(chunks)
            lo, hi = 1 << 60, -1
            for c in range(NC):
                for (w, j, _) in chunks:
                    a = wstarts[c][w] + 128 * j
                    b = min(wstarts[c][w] + 128 * (j + 1), wends[c][w])
                    if a < b:
                        lo = min(lo, int(s_sorted[c][a]))
                        hi = max(hi, int(s_sorted[c][b - 1]))
            assert hi >= 0, (sb, j0)
            assert hi - lo < 32768, (sb, j0, lo, hi)
            p.calls.append(dict(sb=sb, g0=g0, k=k, base=lo,
                                rows=hi - lo + 1, chunks=chunks))
            g0 += k
    assert g0 == p.CT
    return p


def make_core_inputs(p, x, edge_index, t, W1, b1, bn_gamma, bn_beta,
                     W2, b2, ln_gamma, ln_beta):
    x = np.ascontiguousarray(np.asarray(x, np.float32))
    src = np.asarray(edge_index[0]).astype(np.int64)
    dst = np.asarray(edge_index[1]).astype(np.int64)

    iota = np.broadcast_to(np.arange(128, dtype=np.float16), (128, 128)).copy()
    ident = np.eye(128, dtype=np.float32)
    lngh = np.broadcast_to(
        ((1.0 - BETA_L) * np.asarray(ln_gamma, np.float32)).astype(np.float16),
        (128, 128)).copy()
    lnbh = np.broadcast_to(
        ((1.0 - BETA_L) * np.asarray(ln_beta, np.float32)).astype(np.float16),
        (128, 128)).copy()

    tval = float(np.asarray(t))
    vecs = np.zeros((128, 8), np.float32)
    vecs[:, 0] = tval
    vecs[:, 1] = np.asarray(b2, np.float32)
    vecs[:, 2] = np.asarray(bn_gamma, np.float32)[0:128]
    vecs[:, 3] = np.asarray(bn_gamma, np.float32)[128:256]
    vecs[:, 4] = np.asarray(bn_beta, np.float32)[0:128]
    vecs[:, 5] = np.asarray(bn_beta, np.float32)[128:256]
    vecs[:, 6] = EPS_MSG
    vecs[:, 7] = tval * 1e-16

    W1m = np.ascontiguousarray(np.asarray(W1, np.float32).astype(np.float16))
    W2m = np.ascontiguousarray(np.asarray(W2, np.float32).astype(np.float16))

    core = dst // p.NSH
    in_maps = []
    for c in range(NC):
        m = core == c
        s_c = src[m]
        d_c = dst[m] - c * p.NSH
        w_c = d_c // W
        slot_c = (d_c % W).astype(np.float16)
        order = np.lexsort((s_c, w_c))
        s_s, w_s = s_c[order], w_c[order]
        slot_s = slot_c[order]
        wstart = np.searchsorted(w_s, np.arange(p.NW))
        wend = np.searchsorted(w_s, np.arange(p.NW) + 1)

        ids = np.zeros(p.CT * 128, np.int16)
        dstloc = np.full((128, p.CT), -1.0, np.float16)
        for call in p.calls:
            base = call["base"]
            pos = call["g0"] * 128
            for ci, (w, j, _) in enumerate(call["chunks"]):
                g = call["g0"] + ci
                a = wstart[w] + 128 * j
                b = min(wstart[w] + 128 * (j + 1), wend[w])
                nreal = max(0, b - a)
                if nreal > 0:
                    ids[pos:pos + nreal] = (s_s[a:b] - base).astype(np.int16)
                    dstloc[0:nreal, g] = slot_s[a:b]
                pos += 128

        a = ids.reshape(-1, 16).T
        idx_w = np.tile(a, (8, 1)).copy()

        lo_n, hi_n = c * p.NSH, (c + 1) * p.NSH
        im = {
            "x": x,
            "xshard": np.pad(x[lo_n:hi_n], ((0, p.NPAD - p.NSH), (0, 0))),
            "idx": idx_w,
            "dstloc": dstloc,
            "iota": iota,
            "ident": ident,
            "W1f16": W1m,
            "W2f16": W2m,
            "vecs": vecs,
            "lngh": lngh,
            "lnbh": lnbh,
        }
        in_maps.append(im)
    return in_maps


def input_specs(p):
    return {
        "x": ([p.N, D], F32),
        "xshard": ([p.NPAD, D], F32),
        "idx": ([128, p.CT * 8], I16),
        "dstloc": ([128, p.CT], F16),
        "iota": ([128, 128], F16),
        "ident": ([128, 128], F32),
        "W1f16": ([128, 256], F16),
        "W2f16": ([256, 128], F16),
        "vecs": ([128, 8], F32),
        "lngh": ([128, 128], F16),
        "lnbh": ([128, 128], F16),
    }


def emit_kernel(ctx, tc, p, aps):
    nc = tc.nc
    NPAD, NW, NSH = p.NPAD, p.NW, p.NSH
    nsb = (NW + SBW - 1) // SBW

    cpool = ctx.enter_context(tc.tile_pool(name="consts", bufs=1))
    idxt = cpool.tile([128, p.CT * 8], I16, tag="idx")
    nc.sync.dma_start(idxt[:], aps["idx"][:])
    dstloc = cpool.tile([128, p.CT], F16, tag="dstloc")
    nc.sync.dma_start(dstloc[:], aps["dstloc"][:])
    iota = cpool.tile([128, 128], F16, tag="iota")
    nc.sync.dma_start(iota[:], aps["iota"][:])
    ident = cpool.tile([128, 128], F32, tag="ident")
    nc.sync.dma_start(ident[:], aps["ident"][:])
    W1t = cpool.tile([128, 256], F16, tag="w1")
    nc.sync.dma_start(W1t[:], aps["W1f16"][:])
    W2t = [cpool.tile([128, 128], F16, tag=f"w2_{i}", name=f"w2t_{i}")
           for i in range(2)]
    nc.sync.dma_start(W2t[0][:], aps["W2f16"][0:128, :])
    nc.sync.dma_start(W2t[1][:], aps["W2f16"][128:256, :])
    vecs = cpool.tile([128, 8], F32, tag="vecs")
    nc.sync.dma_start(vecs[:], aps["vecs"][:])
    lngh = cpool.tile([128, 128], F16, tag="lngh")
    nc.sync.dma_start(lngh[:], aps["lngh"][:])
    lnbh = cpool.tile([128, 128], F16, tag="lnbh")
    nc.sync.dma_start(lnbh[:], aps["lnbh"][:])
    t_ap = vecs[:, 0:1]
    b2_ap = vecs[:, 1:2]

    xT = cpool.tile([128, NPAD], F32, tag="xT")
    hf = cpool.tile([128, NPAD], F16, tag="hf")

    swpool = tc.tile_pool(name="swp", bufs=1)
    swp = swpool.__enter__()
    swT = swp.tile([128, NW * 256], F32, tag="swT")

    # ---- edge phase ----
    kset = sorted({c["k"] for c in p.calls})
    regs = {k: nc.gpsimd.to_reg(k * 128) for k in kset}

    KW = 2 * SBW
    with tc.tile_pool(name="gat", bufs=6) as gp, \
         tc.tile_pool(name="vals", bufs=3) as vp, \
         tc.tile_pool(name="epsum", bufs=8, space="PSUM") as pp:
        qn = 0
        psw = {}
        sb_calls = {}
        for call in p.calls:
            sb_calls.setdefault(call["sb"], []).append(call)
        for sb in range(nsb):
            for call in sb_calls[sb]:
                k, g0 = call["k"], call["g0"]
                g = gp.tile([128, KW, 128], F32, tag="g")
                nc.gpsimd.dma_gather(
                    g[:, 0:k, :],
                    aps["x"][call["base"]:call["base"] + call["rows"], :],
                    idxt[:, g0 * 8:(g0 + k) * 8],
                    num_idxs=k * 128, num_idxs_reg=regs[k], elem_size=128,
                    single_packet=False, queue_num=qn)
                qn = (qn + 1) % NQ
                gf = g[:, 0:k, :].rearrange("p a b -> p (a b)")
                r = vp.tile([128, KW, 128], F16, tag="r")
                rf = r[:, 0:k, :].rearrange("p a b -> p (a b)")
                nc.scalar.activation(rf, gf, AF.Relu, scale=t_ap)
                e = vp.tile([128, KW, 128], F16, tag="e")
                ef = e[:, 0:k, :].rearrange("p a b -> p (a b)")
                nc.scalar.activation(ef, rf, AF.Exp)
                u = vp.tile([128, KW, 128], F16, tag="u")
                uf = u[:, 0:k, :].rearrange("p a b -> p (a b)")
                nc.vector.tensor_tensor(uf, rf, ef, op=ALU.mult)
                oh = vp.tile([128, KW, 128], F16, tag="oh")
                nc.vector.tensor_tensor(
                    oh[:, 0:k, :],
                    iota[:].unsqueeze(1).broadcast_to([128, k, 128]),
                    dstloc[:, g0:g0 + k].unsqueeze(2).broadcast_to(
                        [128, k, 128]),
                    op=ALU.is_equal)
                for ci, (w, j, last) in enumerate(call["chunks"]):
                    st = j == 0
                    if st:
                        psw[w] = pp.tile([128, 256], F32, tag="ps",
                                         name=f"psw_{w}")
                    nc.tensor.matmul(psw[w][:, 0:128],
                                     e[:, ci, :], oh[:, ci, :],
                                     start=st, stop=last,
                                     skip_group_check=True)
                    nc.tensor.matmul(psw[w][:, 128:256],
                                     u[:, ci, :], oh[:, ci, :],
                                     start=False, stop=last,
                                     skip_group_check=True)
                    if last:
                        nc.scalar.copy(swT[:, w * 256:(w + 1) * 256],
                                       psw[w][:])
                        del psw[w]

    if "dbg_sw" in aps:
        nc.sync.dma_start(aps["dbg_sw"][:], swT[:])

    # ---- xT: transposed (ch-major) x + eps (emitted after the edge loop so
    # its DMAs/transposes fill engine gaps during the edge phase) ----
    with tc.tile_pool(name="tp0", bufs=2, space="PSUM") as tp0, \
         tc.tile_pool(name="sp0", bufs=4) as sp0:
        w = 0
        while w < NW:
            gw = min(4, NW - w)
            ps = tp0.tile([128, 512], F32, tag="pst")
            for i in range(gw):
                xin = sp0.tile([128, 128], F32, tag="xin")
                nc.sync.dma_start(
                    xin[:], aps["xshard"][(w + i) * 128:(w + i + 1) * 128, :])
                nc.tensor.transpose(ps[:, i * 128:(i + 1) * 128], xin[:],
                                    ident[:])
            nc.scalar.activation(xT[:, w * 128:(w + gw) * 128],
                                 ps[:, 0:gw * 128], AF.Identity,
                                 bias=vecs[:, 6:7], scale=1.0)
            w += gw

    # ---- aggregation: agg = u' / (t*(s+1e-16)); hf = agg + x + eps ----
    swv = swT[:].rearrange("p (w q) -> p w q", q=256)
    with tc.tile_pool(name="agg", bufs=1) as agp:
        A = agp.tile([128, NW, 128], F32, tag="A")
        nc.vector.tensor_scalar(A[:], swv[:, :, 0:128], t_ap, vecs[:, 7:8],
                                ALU.mult, ALU.add)
        rcp = agp.tile([128, NW, 128], F32, tag="B")
        nc.vector.reciprocal_approx_fast(rcp[:], A[:])
        nc.vector.tensor_tensor(A[:], swv[:, :, 128:256], rcp[:], op=ALU.mult)
        af = A[:].rearrange("p w q -> p (w q)")
        nc.vector.tensor_tensor(hf[:], af, xT[:], op=ALU.add)

    if "dbg_hf" in aps:
        nc.sync.dma_start(aps["dbg_hf"][:], hf[:])

    # swT no longer needed; free its pool (LIFO: agg pool already closed)
    swpool.__exit__(None, None, None)

    # ---- node phase ----
    np3 = ctx.enter_context(tc.tile_pool(name="node3", bufs=1))
    dramp = ctx.enter_context(tc.tile_pool(name="dram", bufs=1, space="DRAM"))

    with tc.tile_pool(name="tpsum", bufs=2, space="PSUM") as tp, \
         tc.tile_pool(name="scr", bufs=2) as sp:
        # ---- h1 = hf @ W1 (fp16), with BN partial sums via accum_out ----
        h1 = [np3.tile([128, NPAD], F16, tag=f"H{i}", name=f"h1_{i}")
              for i in range(2)]
        ntiles = []
        o = 0
        while o < NPAD:
            ntiles.append((o, min(NT, NPAD - o)))
            o += NT
        nacc = len(ntiles) + 1
        acc = sp.tile([128, 2 * nacc], F32, tag="acc")
        scratch = np3.tile([128, NPAD], F16, tag="Z")
        for ch in (0, 1):
            for i, (o, sz) in enumerate(ntiles):
                psm = tp.tile([128, NT], F32, tag="psmm")
                nc.tensor.matmul(psm[:, 0:sz], W1t[:, ch * 128:(ch + 1) * 128],
                                 hf[:, o:o + sz], start=True, stop=True)
                if o + sz <= NSH:
                    nc.scalar.activation(h1[ch][:, o:o + sz], psm[:, 0:sz],
                                         AF.Identity,
                                         accum_out=acc[:, ch * nacc + i:
                                                       ch * nacc + i + 1])
                else:
                    # split: accumulate only real nodes into BN stats
                    real = NSH - o
                    assert real > 0
                    nc.scalar.activation(h1[ch][:, o:o + real],
                                         psm[:, 0:real], AF.Identity,
                                         accum_out=acc[:, ch * nacc + i:
                                                       ch * nacc + i + 1])
                    nc.scalar.copy(h1[ch][:, o + real:o + sz],
                                   psm[:, real:sz])
            # sum of squares over real nodes (one pass)
            nc.scalar.activation(scratch[:, 0:NSH], h1[ch][:, 0:NSH],
                                 AF.Square,
                                 accum_out=acc[:, ch * nacc + nacc - 1:
                                               ch * nacc + nacc])

        # partials: [sum0, sum1, sumsq0, sumsq1]
        partials = sp.tile([128, 4], F32, tag="partials")
        dump = sp.tile([128, nacc], F32, tag="dump")
        nc.scalar.activation(dump[:, 0:nacc - 1], acc[:, 0:nacc - 1],
                             AF.Identity, accum_out=partials[:, 0:1])
        nc.scalar.activation(dump[:, 0:nacc - 1], acc[:, nacc:2 * nacc - 1],
                             AF.Identity, accum_out=partials[:, 1:2])
        nc.vector.tensor_copy(partials[:, 2:3], acc[:, nacc - 1:nacc])
        nc.vector.tensor_copy(partials[:, 3:4], acc[:, 2 * nacc - 1:2 * nacc])

        ib = dramp.tile([128, 4], F32, tag="ib")
        ob = dramp.tile([128, 4], F32, tag="ob")
        nc.sync.dma_start(ib[:], partials[:])
        nc.gpsimd.collective_compute(
            "AllReduce", ALU.add, replica_groups=[list(range(NC))],
            ins=[ib[:].opt()], outs=[ob[:].opt()])
        gst = sp.tile([128, 4], F32, tag="gst")
        nc.sync.dma_start(gst[:], ob[:])

        mg = sp.tile([128, 2], F32, tag="mg")
        nc.vector.tensor_scalar(mg[:], gst[:, 0:2], 1.0 / N, None, ALU.mult)
        var = sp.tile([128, 2], F32, tag="var")
        nc.vector.tensor_tensor(var[:], mg[:], mg[:], op=ALU.mult)
        ex2 = sp.tile([128, 2], F32, tag="ex2")
        nc.vector.tensor_scalar(ex2[:], gst[:, 2:4], 1.0 / N, None, ALU.mult)
        nc.vector.tensor_tensor(var[:], ex2[:], var[:], op=ALU.subtract)
        nc.vector.tensor_scalar(var[:], var[:], float(BN_EPS), None, ALU.add)
        rcv = sp.tile([128, 2], F32, tag="rcv")
        nc.vector.reciprocal(rcv[:], var[:])
        rstd = sp.tile([128, 2], F32, tag="rstd")
        nc.scalar.sqrt(rstd[:], rcv[:])
        aaf = sp.tile([128, 2], F32, tag="aaf")
        nc.vector.tensor_tensor(aaf[:], vecs[:, 2:4], rstd[:], op=ALU.mult)
        baf = sp.tile([128, 2], F32, tag="baf")
        nc.vector.tensor_tensor(baf[:], mg[:], aaf[:], op=ALU.mult)
        nc.vector.tensor_tensor(baf[:], vecs[:, 4:6], baf[:], op=ALU.subtract)

        for ch in (0, 1):
            nc.scalar.activation(h1[ch][:], h1[ch][:], AF.Relu,
                                 bias=baf[:, ch:ch + 1], scale=aaf[:, ch:ch + 1])

        # ---- y = h1 @ W2 + b2 (ch-major), transpose to node-major ----
        yT = np3.tile([128, NPAD], F32, tag="A")
        for (o, sz) in ntiles:
            psm = tp.tile([128, NT], F32, tag="psy")
            nc.tensor.matmul(psm[:, 0:sz], W2t[0][:], h1[0][:, o:o + sz],
                             start=True, stop=False)
            nc.tensor.matmul(psm[:, 0:sz], W2t[1][:], h1[1][:, o:o + sz],
                             start=False, stop=True)
            nc.scalar.activation(yT[:, o:o + sz], psm[:, 0:sz], AF.Identity,
                                 bias=b2_ap, scale=1.0)

        yN = np3.tile([128, NPAD], F32, tag="B")
        mvall = sp.tile([128, NW * 2], F32, tag="mvall")
        w = 0
        while w < NW:
            gw = min(4, NW - w)
            psm = tp.tile([128, 512], F32, tag="psy")
            for i in range(gw):
                nc.tensor.transpose(psm[:, i * 128:(i + 1) * 128],
                                    yT[:, (w + i) * 128:(w + i + 1) * 128],
                                    ident[:])
            nc.scalar.copy(yN[:, w * 128:(w + gw) * 128], psm[:, 0:gw * 128])
            for i in range(gw):
                st6 = sp.tile([128, 6], F32, tag="st6")
                nc.vector.bn_stats(st6[:],
                                   yN[:, (w + i) * 128:(w + i + 1) * 128])
                nc.vector.bn_aggr(mvall[:, (w + i) * 2:(w + i + 1) * 2],
                                  st6[:])
            w += gw

        mvv = mvall[:].rearrange("p (w q) -> p w q", q=2)
        varn = sp.tile([128, NW], F32, tag="varn")
        nc.vector.tensor_scalar(varn[:], mvv[:, :, 1:2], float(LN_EPS), None,
                                ALU.add)
        rcn = sp.tile([128, NW], F32, tag="rcn")
        nc.vector.reciprocal(rcn[:], varn[:])
        rsn = sp.tile([128, NW], F32, tag="rsn")
        nc.scalar.sqrt(rsn[:], rcn[:])
        nmr = sp.tile([128, NW], F32, tag="nmr")
        nc.vector.tensor_tensor(nmr[:], mvv[:, :, 0:1].rearrange(
            "p w q -> p (w q)"), rsn[:], op=ALU.mult)
        nc.vector.tensor_scalar(nmr[:], nmr[:], -1.0, None, ALU.mult)

        # z = (yN - mu) * rstd per window (ACT per-partition scale/bias)
        Z = np3.tile([128, NPAD], F16, tag="Z")
        for w in range(NW):
            nc.scalar.activation(Z[:, w * 128:(w + 1) * 128],
                                 yN[:, w * 128:(w + 1) * 128], AF.Identity,
                                 bias=nmr[:, w:w + 1], scale=rsn[:, w:w + 1])
        # zh = 0.5*(gamma*z + beta); acc = zh + relu(zh); out = acc + 0.5*x
        zv = Z[:].rearrange("p (w q) -> p w q", q=128)
        nc.vector.tensor_tensor(zv, zv,
                                lngh[:].unsqueeze(1).broadcast_to(
                                    [128, NW, 128]), op=ALU.mult)
        nc.vector.tensor_tensor(zv, zv,
                                lnbh[:].unsqueeze(1).broadcast_to(
                                    [128, NW, 128]), op=ALU.add)
        RL = np3.tile([128, NPAD], F16, tag="R")
        nc.scalar.activation(RL[:], Z[:], AF.Relu)
        nc.vector.tensor_tensor(Z[:], Z[:], RL[:], op=ALU.add)
        xN = np3.tile([128, NPAD], F32, tag="A")
        nc.sync.dma_start(
            xN[:].rearrange("p (w q) -> p w q", q=128),
            aps["xshard"][:].rearrange("(w q) c -> q w c", q=128))
        xh = np3.tile([128, NPAD], F16, tag="R2")
        nc.vector.tensor_scalar(xh[:], xN[:], 1.0 - BETA_L, None, ALU.mult)
        out = np3.tile([128, NPAD], F32, tag="B")
        nc.vector.tensor_tensor(out[:], Z[:], xh[:], op=ALU.add)

        nc.sync.dma_start(
            aps["yout"][:].rearrange("(w q) c -> q w c", q=128),
            out[:].rearrange("p (w q) -> p w q", q=128))


_cache = {}


def _get_compiled(p, compile=True, dbg=False):
    key = (p.key(), compile, dbg)
    if key in _cache:
        return _cache[key]
    nc = bacc.Bacc("TRN2", target_bir_lowering=False, debug=False,
                   num_devices=NC, num_swdge_queues=4)
    aps = {}
    for name, (shape, dt) in input_specs(p).items():
        aps[name] = nc.dram_tensor(name, shape, dt, kind="ExternalInput").ap()
    aps["yout"] = nc.dram_tensor("yout", [p.NPAD, 128], F32,
                                 kind="ExternalOutput").ap()
    if dbg:
        aps["dbg_sw"] = nc.dram_tensor("dbg_sw", [128, p.NW * 256], F32,
                                       kind="ExternalOutput").ap()
        aps["dbg_hf"] = nc.dram_tensor("dbg_hf", [128, p.NPAD], F16,
                                       kind="ExternalOutput").ap()
    with tile.TileContext(nc) as tc:
        with ExitStack() as ctx:
            emit_kernel(ctx, tc, p, aps)
    if compile:
        nc.compile()
    _cache[key] = nc
    return nc


def kernel(x, edge_index, t, W1, b1, bn_gamma, bn_beta, W2, b2,
           ln_gamma, ln_beta):
    x = np.asarray(x)
    edge_index = np.asarray(edge_index)
    p = make_plan(x.shape[0], edge_index)
    ims = make_core_inputs(p, x, edge_index, t, W1, b1, bn_gamma, bn_beta,
                           W2, b2, ln_gamma, ln_beta)
    nc = _get_compiled(p)
    res = bass_utils.run_bass_kernel_spmd(nc, ims, core_ids=list(range(NC)))
    out = np.concatenate([res.results[c]["yout"][:p.NSH] for c in range(NC)])
    return out.astype(np.float32)


# revision 31
# speedup vs baseline: 1.0943x; 1.0495x over previous
"""TRN2 Bass kernel for nn_DeeperGCNLayerMix (GENConv softmax-aggr + MLP/BN/LN mix).

Self-contained: accepts FULL inputs, shards nodes across 8 NeuronCores
internally (SPMD, one NEFF), returns the FULL [50000, 128] output.

v2 strategy (vs v1 baseline):
- Nodes sharded by dst range across 8 cores. Per-core edges bucketed by
  128-node dst window and sorted by src within each window; chunks of 128
  edges. Gather calls are layer-major across a superblock of 7 windows:
  call j gathers the j-th chunk of each window. All chunks in a call share
  one compile-time base row offset into x, with int16 indices relative to
  the base (max span ~16k < 32768) - this removes the lo/hi class split.
- dma_gather descriptor generation is the bottleneck (Q7 core pairs,
  ~8ns/descriptor). Calls round-robin the 4 SWDGE queues, which map to
  disjoint Q7 core pairs, giving ~4x parallel descriptor generation.
  num_idxs registers are materialized once per distinct count so no
  per-call MOVE serializes the gpsimd stream; gather buffers are deep
  (bufs=8) so buffer-reuse waits are pre-satisfied.
- Edge math: r = relu(t*g) (DVE two-op tensor_scalar), e = exp(r) (ACT),
  u = r*e (DVE fp16), one-hot via is_equal (DVE fp16). Two fp16 matmuls
  per chunk accumulate [ch, (s|u)] into a superblock-wide PSUM tile
  [128, 7*256]; one ACT copy per superblock into swT.
- Softmax shift invariance removes the segment-max pass; t is folded into
  r (u' = t*u) and folded back in the denominator reciprocal.
- Node phase ch-major: h = u'/(t*(s+1e-16)) + x + eps, h@W1 (fp16), global
  BatchNorm stats via ACT accum_out sums + AllReduce of [128,4] partials,
  fused affine+relu (ACT), @W2 (fp16), PE transpose back to node-major,
  LayerNorm per node, mixed activation + residual, DMA out.
"""

from contextlib import ExitStack
from dataclasses import dataclass, field

import numpy as np

import concourse.bacc as bacc
import concourse.mybir as mybir
import concourse.tile as tile
from concourse import bass_utils

F32 = mybir.dt.float32
F16 = mybir.dt.float16
I16 = mybir.dt.int16
AF = mybir.ActivationFunctionType
ALU = mybir.AluOpType

N = 50000
NC = 8
D = 128
W = 128
SBW = 7
NT = 512
EPS_MSG = 1e-7
BN_EPS = 1e-5
LN_EPS = 1e-5
BETA_L = 0.5
NQ = 4  # SWDGE queues to round-robin (1..4)


@dataclass
class Plan:
    N: int
    NSH: int = 0
    NW: int = 0
    NPAD: int = 0
    CT: int = 0
    nch: list = field(default_factory=list)
    # calls: list of dicts {sb, j, g0, k, base, rows, chunks:[(w, last)]}
    calls: list = field(default_factory=list)

    def key(self):
        return (self.N, tuple(self.nch),
                tuple((c["g0"], c["k"], c["base"], c["rows"]) for c in self.calls))


def make_plan(n, edge_index):
    src = np.asarray(edge_index[0]).astype(np.int64)
    dst = np.asarray(edge_index[1]).astype(np.int64)
    p = Plan(N=n)
    p.NSH = n // NC
    p.NW = (p.NSH + W - 1) // W
    p.NPAD = p.NW * W

    core = dst // p.NSH
    win = (dst % p.NSH) // W
    cnt = np.zeros((NC, p.NW), np.int64)
    np.add.at(cnt, (core, win), 1)
    nch = np.ceil(cnt / 128).astype(np.int64).max(axis=0)
    nch = np.maximum(nch, 1)
    p.nch = nch.tolist()
    p.CT = int(nch.sum())

    # per-core sorted srcs per window to compute call bases/spans
    wstarts, wends, s_sorted = [], [], []
    for c in range(NC):
        m = core == c
        s_c, w_c = src[m], win[m]
        order = np.lexsort((s_c, w_c))
        s_s, w_s = s_c[order], w_c[order]
        s_sorted.append(s_s)
        wstarts.append(np.searchsorted(w_s, np.arange(p.NW)))
        wends.append(np.searchsorted(w_s, np.arange(p.NW) + 1))

    g0 = 0
    nsb = (p.NW + SBW - 1) // SBW
    for sb in range(nsb):
        ws = list(range(sb * SBW, min((sb + 1) * SBW, p.NW)))
        L = max(nch[w] for w in ws)
        for j0 in range(0, L, 2):
            # merge two chunk layers into one gather call
            chunks = [(w, j, j == nch[w] - 1)
                      for j in (j0, j0 + 1) if j < L
                      for w in ws if j < nch[w]]
            k = len(chunks)
            lo, hi = 1 << 60, -1
            for c in range(NC):
                for (w, j, _) in chunks:
                    a = wstarts[c][w] + 128 * j
                    b = min(wstarts[c][w] + 128 * (j + 1), wends[c][w])
                    if a < b:
                        lo = min(lo, int(s_sorted[c][a]))
                        hi = max(hi, int(s_sorted[c][b - 1]))
            assert hi >= 0, (sb, j0)
            assert hi - lo < 32768, (sb, j0, lo, hi)
            p.calls.append(dict(sb=sb, g0=g0, k=k, base=lo,
                                rows=hi - lo + 1, chunks=chunks))
            g0 += k
    assert g0 == p.CT
    return p


def make_core_inputs(p, x, edge_index, t, W1, b1, bn_gamma, bn_beta,
                     W2, b2, ln_gamma, ln_beta):
    x = np.ascontiguousarray(np.asarray(x, np.float32))
    src = np.asarray(edge_index[0]).astype(np.int64)
    dst = np.asarray(edge_index[1]).astype(np.int64)

    iota = np.broadcast_to(np.arange(128, dtype=np.float16), (128, 128)).copy()
    ident = np.eye(128, dtype=np.float32)
    lngh = np.broadcast_to(
        ((1.0 - BETA_L) * np.asarray(ln_gamma, np.float32)).astype(np.float16),
        (128, 128)).copy()
    lnbh = np.broadcast_to(
        ((1.0 - BETA_L) * np.asarray(ln_beta, np.float32)).astype(np.float16),
        (128, 128)).copy()

    tval = float(np.asarray(t))
    vecs = np.zeros((128, 8), np.float32)
    vecs[:, 0] = tval
    vecs[:, 1] = np.asarray(b2, np.float32)
    vecs[:, 2] = np.asarray(bn_gamma, np.float32)[0:128]
    vecs[:, 3] = np.asarray(bn_gamma, np.float32)[128:256]
    vecs[:, 4] = np.asarray(bn_beta, np.float32)[0:128]
    vecs[:, 5] = np.asarray(bn_beta, np.float32)[128:256]
    vecs[:, 6] = EPS_MSG
    vecs[:, 7] = tval * 1e-16

    W1m = np.ascontiguousarray(np.asarray(W1, np.float32).astype(np.float16))
    W2m = np.ascontiguousarray(np.asarray(W2, np.float32).astype(np.float16))

    core = dst // p.NSH
    in_maps = []
    for c in range(NC):
        m = core == c
        s_c = src[m]
        d_c = dst[m] - c * p.NSH
        w_c = d_c // W
        slot_c = (d_c % W).astype(np.float16)
        order = np.lexsort((s_c, w_c))
        s_s, w_s = s_c[order], w_c[order]
        slot_s = slot_c[order]
        wstart = np.searchsorted(w_s, np.arange(p.NW))
        wend = np.searchsorted(w_s, np.arange(p.NW) + 1)

        ids = np.zeros(p.CT * 128, np.int16)
        dstloc = np.full((128, p.CT), -1.0, np.float16)
        for call in p.calls:
            base = call["base"]
            pos = call["g0"] * 128
            for ci, (w, j, _) in enumerate(call["chunks"]):
                g = call["g0"] + ci
                a = wstart[w] + 128 * j
                b = min(wstart[w] + 128 * (j + 1), wend[w])
                nreal = max(0, b - a)
                if nreal > 0:
                    ids[pos:pos + nreal] = (s_s[a:b] - base).astype(np.int16)
                    dstloc[0:nreal, g] = slot_s[a:b]
                pos += 128

        a = ids.reshape(-1, 16).T
        idx_w = np.tile(a, (8, 1)).copy()

        lo_n, hi_n = c * p.NSH, (c + 1) * p.NSH
        im = {
            "x": x,
            "xshard": np.pad(x[lo_n:hi_n], ((0, p.NPAD - p.NSH), (0, 0))),
            "idx": idx_w,
            "dstloc": dstloc,
            "iota": iota,
            "ident": ident,
            "W1f16": W1m,
            "W2f16": W2m,
            "vecs": vecs,
            "lngh": lngh,
            "lnbh": lnbh,
        }
        in_maps.append(im)
    return in_maps


def input_specs(p):
    return {
        "x": ([p.N, D], F32),
        "xshard": ([p.NPAD, D], F32),
        "idx": ([128, p.CT * 8], I16),
        "dstloc": ([128, p.CT], F16),
        "iota": ([128, 128], F16),
        "ident": ([128, 128], F32),
        "W1f16": ([128, 256], F16),
        "W2f16": ([256, 128], F16),
        "vecs": ([128, 8], F32),
        "lngh": ([128, 128], F16),
        "lnbh": ([128, 128], F16),
    }


def emit_kernel(ctx, tc, p, aps):
    nc = tc.nc
    NPAD, NW, NSH = p.NPAD, p.NW, p.NSH
    nsb = (NW + SBW - 1) // SBW

    cpool = ctx.enter_context(tc.tile_pool(name="consts", bufs=1))
    idxt = cpool.tile([128, p.CT * 8], I16, tag="idx")
    nc.sync.dma_start(idxt[:], aps["idx"][:])
    dstloc = cpool.tile([128, p.CT], F16, tag="dstloc")
    nc.sync.dma_start(dstloc[:], aps["dstloc"][:])
    iota = cpool.tile([128, 128], F16, tag="iota")
    nc.sync.dma_start(iota[:], aps["iota"][:])
    ident = cpool.tile([128, 128], F32, tag="ident")
    nc.sync.dma_start(ident[:], aps["ident"][:])
    W1t = cpool.tile([128, 256], F16, tag="w1")
    nc.sync.dma_start(W1t[:], aps["W1f16"][:])
    W2t = [cpool.tile([128, 128], F16, tag=f"w2_{i}", name=f"w2t_{i}")
           for i in range(2)]
    nc.sync.dma_start(W2t[0][:], aps["W2f16"][0:128, :])
    nc.sync.dma_start(W2t[1][:], aps["W2f16"][128:256, :])
    vecs = cpool.tile([128, 8], F32, tag="vecs")
    nc.sync.dma_start(vecs[:], aps["vecs"][:])
    lngh = cpool.tile([128, 128], F16, tag="lngh")
    nc.sync.dma_start(lngh[:], aps["lngh"][:])
    lnbh = cpool.tile([128, 128], F16, tag="lnbh")
    nc.sync.dma_start(lnbh[:], aps["lnbh"][:])
    t_ap = vecs[:, 0:1]
    b2_ap = vecs[:, 1:2]

    xT = cpool.tile([128, NPAD], F32, tag="xT")
    hf = cpool.tile([128, NPAD], F16, tag="hf")

    swpool = tc.tile_pool(name="swp", bufs=1)
    swp = swpool.__enter__()
    swT = swp.tile([128, NW * 256], F32, tag="swT")

    # xT build pools stay OPEN across the edge phase: closing them before the
    # edge pools would add a pool-boundary dependency that serializes the
    # whole xT build ahead of the first gather (~66us ramp).
    xtp_cm = tc.tile_pool(name="xtp", bufs=1, space="PSUM")
    xtp = xtp_cm.__enter__()
    xts_cm = tc.tile_pool(name="xts", bufs=4)
    xts = xts_cm.__enter__()

    def emit_xt_build():
        w = 0
        while w < NW:
            gw = min(4, NW - w)
            ps = xtp.tile([128, 512], F32, tag="pst")
            for i in range(gw):
                xin = xts.tile([128, 128], F32, tag="xin")
                nc.sync.dma_start(
                    xin[:], aps["xshard"][(w + i) * 128:(w + i + 1) * 128, :])
                nc.tensor.transpose(ps[:, i * 128:(i + 1) * 128], xin[:],
                                    ident[:])
            nc.scalar.activation(xT[:, w * 128:(w + gw) * 128],
                                 ps[:, 0:gw * 128], AF.Identity,
                                 bias=vecs[:, 6:7], scale=1.0)
            w += gw

    # ---- edge phase ----
    kset = sorted({c["k"] for c in p.calls})
    regs = {k: nc.gpsimd.to_reg(k * 128) for k in kset}

    KW = 2 * SBW
    with tc.tile_pool(name="gat", bufs=6) as gp, \
         tc.tile_pool(name="vals", bufs=3) as vp, \
         tc.tile_pool(name="epsum", bufs=7, space="PSUM") as pp:
        qn = 0
        psw = {}
        sb_calls = {}
        for call in p.calls:
            sb_calls.setdefault(call["sb"], []).append(call)
        for sb in range(nsb):
            for call in sb_calls[sb]:
                k, g0 = call["k"], call["g0"]
                g = gp.tile([128, KW, 128], F32, tag="g")
                nc.gpsimd.dma_gather(
                    g[:, 0:k, :],
                    aps["x"][call["base"]:call["base"] + call["rows"], :],
                    idxt[:, g0 * 8:(g0 + k) * 8],
                    num_idxs=k * 128, num_idxs_reg=regs[k], elem_size=128,
                    single_packet=False, queue_num=qn)
                qn = (qn + 1) % NQ
                gf = g[:, 0:k, :].rearrange("p a b -> p (a b)")
                r = vp.tile([128, KW, 128], F16, tag="r")
                rf = r[:, 0:k, :].rearrange("p a b -> p (a b)")
                nc.scalar.activation(rf, gf, AF.Relu, scale=t_ap)
                e = vp.tile([128, KW, 128], F16, tag="e")
                ef = e[:, 0:k, :].rearrange("p a b -> p (a b)")
                nc.scalar.activation(ef, rf, AF.Exp)
                u = vp.tile([128, KW, 128], F16, tag="u")
                uf = u[:, 0:k, :].rearrange("p a b -> p (a b)")
                nc.vector.tensor_tensor(uf, rf, ef, op=ALU.mult)
                oh = vp.tile([128, KW, 128], F16, tag="oh")
                nc.vector.tensor_tensor(
                    oh[:, 0:k, :],
                    iota[:].unsqueeze(1).broadcast_to([128, k, 128]),
                    dstloc[:, g0:g0 + k].unsqueeze(2).broadcast_to(
                        [128, k, 128]),
                    op=ALU.is_equal)
                for ci, (w, j, last) in enumerate(call["chunks"]):
                    st = j == 0
                    if st:
                        psw[w] = pp.tile([128, 256], F32, tag="ps",
                                         name=f"psw_{w}")
                    nc.tensor.matmul(psw[w][:, 0:128],
                                     e[:, ci, :], oh[:, ci, :],
                                     start=st, stop=last,
                                     skip_group_check=True)
                    nc.tensor.matmul(psw[w][:, 128:256],
                                     u[:, ci, :], oh[:, ci, :],
                                     start=False, stop=last,
                                     skip_group_check=True)
                    if last:
                        nc.scalar.copy(swT[:, w * 256:(w + 1) * 256],
                                       psw[w][:])
                        del psw[w]
            if sb == 0:
                emit_xt_build()

    if "dbg_sw" in aps:
        nc.sync.dma_start(aps["dbg_sw"][:], swT[:])

    xts_cm.__exit__(None, None, None)
    xtp_cm.__exit__(None, None, None)

    # ---- aggregation: agg = u' / (t*(s+1e-16)); hf = agg + x + eps ----
    swv = swT[:].rearrange("p (w q) -> p w q", q=256)
    with tc.tile_pool(name="agg", bufs=1) as agp:
        A = agp.tile([128, NW, 128], F32, tag="A")
        nc.vector.tensor_scalar(A[:], swv[:, :, 0:128], t_ap, vecs[:, 7:8],
                                ALU.mult, ALU.add)
        rcp = agp.tile([128, NW, 128], F32, tag="B")
        nc.vector.reciprocal_approx_fast(rcp[:], A[:])
        nc.vector.tensor_tensor(A[:], swv[:, :, 128:256], rcp[:], op=ALU.mult)
        af = A[:].rearrange("p w q -> p (w q)")
        nc.vector.tensor_tensor(hf[:], af, xT[:], op=ALU.add)

    if "dbg_hf" in aps:
        nc.sync.dma_start(aps["dbg_hf"][:], hf[:])

    # swT no longer needed; free its pool (LIFO: agg pool already closed)
    swpool.__exit__(None, None, None)

    # ---- node phase ----
    np3 = ctx.enter_context(tc.tile_pool(name="node3", bufs=1))
    dramp = ctx.enter_context(tc.tile_pool(name="dram", bufs=1, space="DRAM"))

    with tc.tile_pool(name="tpsum", bufs=2, space="PSUM") as tp, \
         tc.tile_pool(name="scr", bufs=2) as sp:
        # ---- h1 = hf @ W1 (fp16), with BN partial sums via accum_out ----
        h1 = [np3.tile([128, NPAD], F16, tag=f"H{i}", name=f"h1_{i}")
              for i in range(2)]
        ntiles = []
        o = 0
        while o < NPAD:
            ntiles.append((o, min(NT, NPAD - o)))
            o += NT
        nacc = len(ntiles) + 1
        acc = sp.tile([128, 2 * nacc], F32, tag="acc")
        scratch = np3.tile([128, NPAD], F16, tag="Z")
        for ch in (0, 1):
            for i, (o, sz) in enumerate(ntiles):
                psm = tp.tile([128, NT], F32, tag="psmm")
                nc.tensor.matmul(psm[:, 0:sz], W1t[:, ch * 128:(ch + 1) * 128],
                                 hf[:, o:o + sz], start=True, stop=True)
                if o + sz <= NSH:
                    nc.scalar.activation(h1[ch][:, o:o + sz], psm[:, 0:sz],
                                         AF.Identity,
                                         accum_out=acc[:, ch * nacc + i:
                                                       ch * nacc + i + 1])
                else:
                    # split: accumulate only real nodes into BN stats
                    real = NSH - o
                    assert real > 0
                    nc.scalar.activation(h1[ch][:, o:o + real],
                                         psm[:, 0:real], AF.Identity,
                                         accum_out=acc[:, ch * nacc + i:
                                                       ch * nacc + i + 1])
                    nc.scalar.copy(h1[ch][:, o + real:o + sz],
                                   psm[:, real:sz])
            # sum of squares over real nodes (one pass)
            nc.scalar.activation(scratch[:, 0:NSH], h1[ch][:, 0:NSH],
                                 AF.Square,
                                 accum_out=acc[:, ch * nacc + nacc - 1:
                                               ch * nacc + nacc])

        # partials: [sum0, sum1, sumsq0, sumsq1]
        partials = sp.tile([128, 4], F32, tag="partials")
        dump = sp.tile([128, nacc], F32, tag="dump")
        nc.scalar.activation(dump[:, 0:nacc - 1], acc[:, 0:nacc - 1],
                             AF.Identity, accum_out=partials[:, 0:1])
        nc.scalar.activation(dump[:, 0:nacc - 1], acc[:, nacc:2 * nacc - 1],
                             AF.Identity, accum_out=partials[:, 1:2])
        nc.vector.tensor_copy(partials[:, 2:3], acc[:, nacc - 1:nacc])
        nc.vector.tensor_copy(partials[:, 3:4], acc[:, 2 * nacc - 1:2 * nacc])

        ib = dramp.tile([128, 4], F32, tag="ib")
        ob = dramp.tile([128, 4], F32, tag="ob")
        nc.sync.dma_start(ib[:], partials[:])
        nc.gpsimd.collective_compute(
            "AllReduce", ALU.add, replica_groups=[list(range(NC))],
            ins=[ib[:].opt()], outs=[ob[:].opt()])
        gst = sp.tile([128, 4], F32, tag="gst")
        nc.sync.dma_start(gst[:], ob[:])

        mg = sp.tile([128, 2], F32, tag="mg")
        nc.vector.tensor_scalar(mg[:], gst[:, 0:2], 1.0 / N, None, ALU.mult)
        var = sp.tile([128, 2], F32, tag="var")
        nc.vector.tensor_tensor(var[:], mg[:], mg[:], op=ALU.mult)
        ex2 = sp.tile([128, 2], F32, tag="ex2")
        nc.vector.tensor_scalar(ex2[:], gst[:, 2:4], 1.0 / N, None, ALU.mult)
        nc.vector.tensor_tensor(var[:], ex2[:], var[:], op=ALU.subtract)
        nc.vector.tensor_scalar(var[:], var[:], float(BN_EPS), None, ALU.add)
        rcv = sp.tile([128, 2], F32, tag="rcv")
        nc.vector.reciprocal(rcv[:], var[:])
        rstd = sp.tile([128, 2], F32, tag="rstd")
        nc.scalar.sqrt(rstd[:], rcv[:])
        aaf = sp.tile([128, 2], F32, tag="aaf")
        nc.vector.tensor_tensor(aaf[:], vecs[:, 2:4], rstd[:], op=ALU.mult)
        baf = sp.tile([128, 2], F32, tag="baf")
        nc.vector.tensor_tensor(baf[:], mg[:], aaf[:], op=ALU.mult)
        nc.vector.tensor_tensor(baf[:], vecs[:, 4:6], baf[:], op=ALU.subtract)

        for ch in (0, 1):
            nc.scalar.activation(h1[ch][:], h1[ch][:], AF.Relu,
                                 bias=baf[:, ch:ch + 1], scale=aaf[:, ch:ch + 1])

        # ---- y = h1 @ W2 + b2 (ch-major), transpose to node-major ----
        yT = np3.tile([128, NPAD], F32, tag="A")
        for (o, sz) in ntiles:
            psm = tp.tile([128, NT], F32, tag="psy")
            nc.tensor.matmul(psm[:, 0:sz], W2t[0][:], h1[0][:, o:o + sz],
                             start=True, stop=False)
            nc.tensor.matmul(psm[:, 0:sz], W2t[1][:], h1[1][:, o:o + sz],
                             start=False, stop=True)
            nc.scalar.activation(yT[:, o:o + sz], psm[:, 0:sz], AF.Identity,
                                 bias=b2_ap, scale=1.0)

        yN = np3.tile([128, NPAD], F32, tag="B")
        mvall = sp.tile([128, NW * 2], F32, tag="mvall")
        w = 0
        while w < NW:
            gw = min(4, NW - w)
            psm = tp.tile([128, 512], F32, tag="psy")
            for i in range(gw):
                nc.tensor.transpose(psm[:, i * 128:(i + 1) * 128],
                                    yT[:, (w + i) * 128:(w + i + 1) * 128],
                                    ident[:])
            nc.scalar.copy(yN[:, w * 128:(w + gw) * 128], psm[:, 0:gw * 128])
            for i in range(gw):
                st6 = sp.tile([128, 6], F32, tag="st6")
                nc.vector.bn_stats(st6[:],
                                   yN[:, (w + i) * 128:(w + i + 1) * 128])
                nc.vector.bn_aggr(mvall[:, (w + i) * 2:(w + i + 1) * 2],
                                  st6[:])
            w += gw

        mvv = mvall[:].rearrange("p (w q) -> p w q", q=2)
        varn = sp.tile([128, NW], F32, tag="varn")
        nc.vector.tensor_scalar(varn[:], mvv[:, :, 1:2], float(LN_EPS), None,
                                ALU.add)
        rcn = sp.tile([128, NW], F32, tag="rcn")
        nc.vector.reciprocal(rcn[:], varn[:])
        rsn = sp.tile([128, NW], F32, tag="rsn")
        nc.scalar.sqrt(rsn[:], rcn[:])
        nmr = sp.tile([128, NW], F32, tag="nmr")
        nc.vector.tensor_tensor(nmr[:], mvv[:, :, 0:1].rearrange(
            "p w q -> p (w q)"), rsn[:], op=ALU.mult)
        nc.vector.tensor_scalar(nmr[:], nmr[:], -1.0, None, ALU.mult)

        # z = (yN - mu) * rstd per window (ACT per-partition scale/bias)
        Z = np3.tile([128, NPAD], F16, tag="Z")
        for w in range(NW):
            nc.scalar.activation(Z[:, w * 128:(w + 1) * 128],
                                 yN[:, w * 128:(w + 1) * 128], AF.Identity,
                                 bias=nmr[:, w:w + 1], scale=rsn[:, w:w + 1])
        # zh = 0.5*(gamma*z + beta); acc = zh + relu(zh); out = acc + 0.5*x
        zv = Z[:].rearrange("p (w q) -> p w q", q=128)
        nc.vector.tensor_tensor(zv, zv,
                                lngh[:].unsqueeze(1).broadcast_to(
                                    [128, NW, 128]), op=ALU.mult)
        nc.vector.tensor_tensor(zv, zv,
                                lnbh[:].unsqueeze(1).broadcast_to(
                                    [128, NW, 128]), op=ALU.add)
        RL = np3.tile([128, NPAD], F16, tag="R")
        nc.scalar.activation(RL[:], Z[:], AF.Relu)
        nc.vector.tensor_tensor(Z[:], Z[:], RL[:], op=ALU.add)
        xN = np3.tile([128, NPAD], F32, tag="A")
        nc.sync.dma_start(
            xN[:].rearrange("p (w q) -> p w q", q=128),
            aps["xshard"][:].rearrange("(w q) c -> q w c", q=128))
        xh = np3.tile([128, NPAD], F16, tag="R2")
        nc.vector.tensor_scalar(xh[:], xN[:], 1.0 - BETA_L, None, ALU.mult)
        out = np3.tile([128, NPAD], F32, tag="B")
        nc.vector.tensor_tensor(out[:], Z[:], xh[:], op=ALU.add)

        nc.sync.dma_start(
            aps["yout"][:].rearrange("(w q) c -> q w c", q=128),
            out[:].rearrange("p (w q) -> p w q", q=128))


_cache = {}


def _get_compiled(p, compile=True, dbg=False):
    key = (p.key(), compile, dbg)
    if key in _cache:
        return _cache[key]
    nc = bacc.Bacc("TRN2", target_bir_lowering=False, debug=False,
                   num_devices=NC, num_swdge_queues=4)
    aps = {}
    for name, (shape, dt) in input_specs(p).items():
        aps[name] = nc.dram_tensor(name, shape, dt, kind="ExternalInput").ap()
    aps["yout"] = nc.dram_tensor("yout", [p.NPAD, 128], F32,
                                 kind="ExternalOutput").ap()
    if dbg:
        aps["dbg_sw"] = nc.dram_tensor("dbg_sw", [128, p.NW * 256], F32,
                                       kind="ExternalOutput").ap()
        aps["dbg_hf"] = nc.dram_tensor("dbg_hf", [128, p.NPAD], F16,
                                       kind="ExternalOutput").ap()
    with tile.TileContext(nc) as tc:
        with ExitStack() as ctx:
            emit_kernel(ctx, tc, p, aps)
    if compile:
        nc.compile()
    _cache[key] = nc
    return nc


def kernel(x, edge_index, t, W1, b1, bn_gamma, bn_beta, W2, b2,
           ln_gamma, ln_beta):
    x = np.asarray(x)
    edge_index = np.asarray(edge_index)
    p = make_plan(x.shape[0], edge_index)
    ims = make_core_inputs(p, x, edge_index, t, W1, b1, bn_gamma, bn_beta,
                           W2, b2, ln_gamma, ln_beta)
    nc = _get_compiled(p)
    res = bass_utils.run_bass_kernel_spmd(nc, ims, core_ids=list(range(NC)))
    out = np.concatenate([res.results[c]["yout"][:p.NSH] for c in range(NC)])
    return out.astype(np.float32)
